# revision 23
# baseline (speedup 1.0000x reference)
"""AttentionBlock (GroupNorm + single-head spatial attention + proj + residual)
on 8 trn2 NeuronCores, data-parallel over the batch (1 image per core).

Full inputs in, full outputs out. v2: all GEMMs run in fp8e4m3 with DoubleRow
perf mode (2 contraction rows/cycle), and the proj matmul is fused into the
value projection host-side:

    y = x + proj_b' + (W_pv xn) E D^-1,   W_pv = proj_w @ W_v,
    E = exp(scale * xn^T A xn),           A = W_q^T W_k  (q/k fold),
    D = colsum(E).

Bias handling is exact for arbitrary qkv_b: the v-bias folds through the
attention (rows sum to 1) into proj_b'; the q-bias contributes a per-key
logit term w0^T xn_j with w0 = W_k^T q_b, added as the per-partition bias of
the t-evacuation; the k-bias only produces per-query logit shifts, which
softmax cancels.

fp8 precision (measured host-side vs the fp32 reference): rel err ~4.5e-3,
dominated by the E/value quantization; the residual branch is only ~7% of
the output norm. A and W_pv are pre-scaled by 16 to dodge the fp8 subnormal
cutoff; the 1/16 rides the PSUM-evacuation activation scale.

Layouts (prepared host-side so every DMA is contiguous, no device transposes):
  - activations [128 part, ct, pix] (channel tiles of 128)
  - qw = [A*16 | W_pv^T*16] as [c_in part, ct_in, 2C] fp8
  - u is produced transposed (u^T = xn^T W_pv^T) so the attention contraction
    over pixels has pixels on partitions everywhere.
"""

import sys

sys.path.insert(0, "/opt/trn_rl_repo")

import numpy as np
import ml_dtypes

import concourse.bass as bass
import concourse.tile as tile
from concourse import bacc, mybir
from concourse.bass_utils import run_bass_kernel_spmd
from concourse.tile_rust import add_dep_helper

F32 = mybir.dt.float32
FP8 = mybir.dt.float8e4
NPF8 = ml_dtypes.float8_e4m3
DR = mybir.MatmulPerfMode.DoubleRow

C = 512          # channels
NPIX = 1024      # pixels per image (32*32)
CT = 4           # channel tiles of 128
JT = 8           # pixel tiles of 128
NH = 2           # halves of NPIX for the 512-wide moving dim
G = 32           # groups
GS = 16          # channels per group
EPS = 1e-5
SCALE = C ** -0.5
WSCALE = 16.0    # host pre-scale on A / W_pv to stay clear of fp8 subnormals

WARM_MMS = 27    # PE warm-up matmuls covering the input-DMA + GN-stats window
FILL1 = 5        # keep-warm matmuls between the GN group-sum and broadcast
FILL2 = 13       # keep-warm matmuls covering the xn-quantize window

TRACE = False          # set True (from test.py) to capture an NTFF profile
TRACE_KW = {}          # extra kwargs for run_bass_kernel_spmd
LAST_RESULTS = None    # BassKernelResults of the most recent run

_cache = {}


def _build():
    nc = bacc.Bacc("TRN2")

    # x ships as bf16: it only feeds the GN stats and the fp8 quantize;
    # the residual rides xpbt. Halves the critical input-DMA window.
    x_d = nc.dram_tensor("x", [128, CT, NPIX], mybir.dt.bfloat16, kind="ExternalInput")
    qw_d = nc.dram_tensor("qw", [128, CT, 2 * C], FP8, kind="ExternalInput")
    # packed per-channel columns: gnw, gnb, w0
    cols_d = nc.dram_tensor("cols", [128, CT, 3], F32, kind="ExternalInput")
    # x^T + pb, [pix part, jt, c] — the residual in the output layout (bf16)
    BF16 = mybir.dt.bfloat16
    xpbt_d = nc.dram_tensor("xpbt", [128, JT, C], BF16, kind="ExternalInput")
    # y^T [pix part, jt, c]; host transposes back
    y_d = nc.dram_tensor("y", [128, JT, C], F32, kind="ExternalOutput")

    # Indicator constants for the cross-partition group reductions.
    # ind1[p, ct*G + g] = 1 iff channel (ct*128+p) belongs to group g.
    ind1 = np.zeros((128, CT * G), np.float32)
    for ct in range(CT):
        for p in range(128):
            ind1[p, ct * G + ct * 8 + p // GS] = 1.0
    # ind2[g, c] = 1 iff channel c belongs to group g.
    ind2 = np.zeros((G, C), np.float32)
    for c in range(C):
        ind2[c // GS, c] = 1.0
    ind1_d = nc.inline_tensor(ind1, name="ind1")
    ind2_d = nc.inline_tensor(ind2, name="ind2")

    with tile.TileContext(nc) as tc:
        with (
            nc.allow_low_precision(reason="fp8 matmuls, fp8 activation stores"),
            tc.tile_pool(name="persist", bufs=1) as pers,
            tc.tile_pool(name="small", bufs=4) as spool,
            tc.tile_pool(name="att", bufs=4) as apool,
            tc.tile_pool(name="ps", bufs=8, space="PSUM") as psp,
        ):
            # ---- x, one DMA per (ct, half); issues split across the two
            # HWDGE-capable queues (sync + scalar) so more DMA rings engage.
            x_sb = pers.tile([128, CT, NPIX], mybir.dt.bfloat16)
            x_dmas = []
            for ct in range(CT):
                for nh in range(NH):
                    eng = nc.sync if (ct * NH + nh) % 2 == 0 else nc.scalar
                    x_dmas.append(
                        eng.dma_start(
                            x_sb[:, ct, nh * 512 : (nh + 1) * 512],
                            x_d[:, ct, nh * 512 : (nh + 1) * 512],
                        )
                    )

            # ---- small loads (after x in the issue queue) ------------------
            cols_sb = pers.tile([128, CT, 3], F32)
            nc.sync.dma_start(cols_sb[:], cols_d[:])
            ind1_sb = pers.tile([128, CT * G], F32)
            nc.sync.dma_start(ind1_sb[:], ind1_d[:])
            ind2_sb = pers.tile([G, C], F32)
            nc.sync.dma_start(ind2_sb[:], ind2_d[:])

            # ---- weights + residual, serialized behind x -------------------
            qw_sb = pers.tile([128, CT, 2 * C], FP8)
            d = nc.sync.dma_start(qw_sb[:], qw_d[:])
            add_dep_helper(d.ins, x_dmas[-1].ins, sync=True,
                           reason="let x DMA finish first")
            xpbt_sb = pers.tile([128, JT, C], BF16)
            d = nc.sync.dma_start(xpbt_sb[:], xpbt_d[:])
            add_dep_helper(d.ins, x_dmas[-1].ins, sync=True,
                           reason="let x DMA finish first")

            gnw_sb = cols_sb[:, :, 0]
            gnb_sb = cols_sb[:, :, 1]
            w0_sb = cols_sb[:, :, 2]

            # ---- constants from memsets (no DMA dependency) ----------------
            warm8 = pers.tile([128, 2, 512], FP8)
            nc.gpsimd.memset(warm8[:], 1.0)
            eps_sb = pers.tile([G, 1], F32)
            nc.vector.memset(eps_sb[:], EPS)
            # pre-load the Exp activation table during the DMA window so the
            # scalar engine doesn't stall the S phase with a mid-stream load
            dume = pers.tile([1, 1], FP8)
            nc.scalar.activation(
                dume[:], eps_sb[0:1, 0:1], mybir.ActivationFunctionType.Exp,
                scale=SCALE / WSCALE,
            )

            def warm(n):
                for _ in range(n):
                    wps = psp.tile([128, 512], F32, tag="ps")
                    nc.tensor.matmul(
                        wps[:], warm8[:, :, 0:128], warm8[:], start=True,
                        stop=True, perf_mode=DR,
                    )

            # ---- PE warm-up: ramp the clock while the input DMAs stream ----
            warm(WARM_MMS)

            # ---- group norm ------------------------------------------------
            # per-channel mean / E[x^2] along pixels, then group-combine via
            # indicator matmuls (contraction over the partition dim).
            statcols = pers.tile([128, CT, 2], F32)
            for ct in range(CT):
                st6 = spool.tile([128, 2, 6], F32, tag="st6")
                nc.vector.bn_stats(st6[:, 0, :], x_sb[:, ct, 0:512])
                nc.vector.bn_stats(st6[:, 1, :], x_sb[:, ct, 512:1024])
                mv = spool.tile([128, 2], F32, tag="mv")
                nc.vector.bn_aggr(mv[:], st6[:])
                nc.vector.tensor_copy(statcols[:, ct, 0:1], mv[:, 0:1])
                # E[x^2] = var + mean^2
                nc.vector.tensor_mul(statcols[:, ct, 1:2], mv[:, 0:1], mv[:, 0:1])
                nc.vector.tensor_add(
                    statcols[:, ct, 1:2], statcols[:, ct, 1:2], mv[:, 1:2]
                )

            gsum_ps = psp.tile([G, 2], F32, tag="ps")
            for ct in range(CT):
                nc.tensor.matmul(
                    gsum_ps[:],
                    ind1_sb[:, ct * G : (ct + 1) * G],
                    statcols[:, ct, :],
                    start=(ct == 0),
                    stop=(ct == CT - 1),
                )
            warm(FILL1)
            gs_sb = spool.tile([G, 2], F32, tag="gs")
            nc.vector.tensor_scalar_mul(gs_sb[:], gsum_ps[:], 1.0 / GS)
            var32 = spool.tile([G, 1], F32, tag="var32")
            nc.vector.tensor_mul(var32[:], gs_sb[:, 0:1], gs_sb[:, 0:1])
            nc.vector.tensor_sub(var32[:], gs_sb[:, 1:2], var32[:])
            # grow = [rstd, mean * rstd] per group
            grow = pers.tile([G, 2], F32)
            nc.scalar.activation(
                grow[:, 0:1],
                var32[:],
                mybir.ActivationFunctionType.Sqrt,
                bias=eps_sb[:],
            )
            nc.vector.reciprocal(grow[:, 0:1], grow[:, 0:1])
            nc.vector.tensor_mul(grow[:, 1:2], gs_sb[:, 0:1], grow[:, 0:1])

            # broadcast group stats back to channels; fold gn weight/bias into
            # per-channel scale A and bias B:  xn = x*A + B
            chsb = pers.tile([128, CT, 2], F32)
            for ct in range(CT):
                bc_ps = psp.tile([128, 2], F32, tag="ps")
                nc.tensor.matmul(
                    bc_ps[:],
                    ind2_sb[:, ct * 128 : (ct + 1) * 128],
                    grow[:],
                    start=True,
                    stop=True,
                )
                nc.vector.tensor_mul(
                    chsb[:, ct, 0:1], gnw_sb[:, ct : ct + 1], bc_ps[:, 0:1]
                )
                nc.vector.tensor_mul(
                    chsb[:, ct, 1:2], gnw_sb[:, ct : ct + 1], bc_ps[:, 1:2]
                )
                nc.vector.tensor_sub(
                    chsb[:, ct, 1:2], gnb_sb[:, ct : ct + 1], chsb[:, ct, 1:2]
                )
            warm(FILL2)

            # ---- xn in fp8, nh-major so the first GEMMs start early.
            # Split vector/pool: both halves in flight at once.
            xn8 = pers.tile([128, CT, NPIX], FP8)
            for nh in range(NH):
                for ct in range(CT):
                    sl = (slice(None), ct, slice(nh * 512, (nh + 1) * 512))
                    eng = nc.vector if (ct % 2 == 0) else nc.gpsimd
                    eng.tensor_scalar(
                        out=xn8[sl],
                        in0=x_sb[sl],
                        scalar1=chsb[:, ct, 0:1],
                        scalar2=chsb[:, ct, 1:2],
                        op0=mybir.AluOpType.mult,
                        op1=mybir.AluOpType.add,
                    )

            # ---- t = A^T xn (+ w0), fp8; evac on scalar --------------------
            t8 = pers.tile([128, CT, NPIX], FP8)
            for nh in range(NH):
                for co in range(CT):
                    ps = psp.tile([128, 512], F32, tag="ps")
                    for cp in range(2):
                        nc.tensor.matmul(
                            ps[:],
                            qw_sb[:, 2 * cp : 2 * cp + 2, co * 128 : (co + 1) * 128],
                            xn8[:, 2 * cp : 2 * cp + 2, nh * 512 : (nh + 1) * 512],
                            start=(cp == 0),
                            stop=(cp == 1),
                            perf_mode=DR,
                        )
                    nc.vector.tensor_scalar(
                        out=t8[:, co, nh * 512 : (nh + 1) * 512],
                        in0=ps[:],
                        scalar1=w0_sb[:, co : co + 1],
                        scalar2=None,
                        op0=mybir.AluOpType.add,
                    )

            e8 = pers.tile([128, JT, NPIX], FP8)
            rcol_sb = pers.tile([128, JT], F32)
            u8 = pers.tile([128, JT, C], FP8)

            # The scalar engine (24 exp/copy evacuations) paces the t/S
            # phases, so u, den and out matmuls are interleaved into the S
            # stream to keep the PE busy during scalar waits.

            def s_tile(jt, nh):
                # S^T = xn^T t (pix_j on partitions), E = exp(scale * S^T)
                ps = psp.tile([128, 512], F32, tag="ps")
                for cp in range(2):
                    nc.tensor.matmul(
                        ps[:],
                        xn8[:, 2 * cp : 2 * cp + 2, jt * 128 : (jt + 1) * 128],
                        t8[:, 2 * cp : 2 * cp + 2, nh * 512 : (nh + 1) * 512],
                        start=(cp == 0),
                        stop=(cp == 1),
                        perf_mode=DR,
                    )
                nc.scalar.activation(
                    e8[:, jt, nh * 512 : (nh + 1) * 512],
                    ps[:],
                    mybir.ActivationFunctionType.Exp,
                    scale=SCALE / WSCALE,
                )

            def u_tile(jt):
                # u^T = xn^T W_pv^T, fp8; evac on vector
                ps = psp.tile([128, 512], F32, tag="ps")
                for cp in range(2):
                    nc.tensor.matmul(
                        ps[:],
                        xn8[:, 2 * cp : 2 * cp + 2, jt * 128 : (jt + 1) * 128],
                        qw_sb[:, 2 * cp : 2 * cp + 2, C : 2 * C],
                        start=(cp == 0),
                        stop=(cp == 1),
                        perf_mode=DR,
                    )
                nc.vector.tensor_scalar_mul(u8[:, jt, :], ps[:], 1.0 / WSCALE)

            def den_col(it):
                # den[i] = sum_j E[j, i] as a per-partition column: E-tile as
                # stationary, fp8 ones as the (2-row) moving operand.
                dps = psp.tile([128, 1], F32, name=f"den{it}", tag="ps")
                for p in range(4):
                    nc.tensor.matmul(
                        dps[:],
                        e8[:, 2 * p : 2 * p + 2, it * 128 : (it + 1) * 128],
                        warm8[:, 0:2, 0:1],
                        start=(p == 0),
                        stop=(p == 3),
                        perf_mode=DR,
                    )
                rscr = spool.tile([128, 1], F32, tag="rscr")
                nc.vector.reciprocal_approx_accurate(
                    rcol_sb[:, it : it + 1], dps[:], rscr[:]
                )

            def out_tile(it):
                # out^T = (u E)^T D^-1 + (x^T + pb): E-tile stationary puts
                # queries on partitions, so 1/den is a per-partition scalar
                # on the (single, fused) vector evacuation.
                ps = psp.tile([128, 512], F32, tag="ps")
                for p in range(4):
                    nc.tensor.matmul(
                        ps[:],
                        e8[:, 2 * p : 2 * p + 2, it * 128 : (it + 1) * 128],
                        u8[:, 2 * p : 2 * p + 2, :],
                        start=(p == 0),
                        stop=(p == 3),
                        perf_mode=DR,
                    )
                yt = apool.tile([128, 512], F32, tag="yt")
                if it % 2 == 1:
                    nc.vector.scalar_tensor_tensor(
                        out=yt[:],
                        in0=ps[:],
                        scalar=rcol_sb[:, it : it + 1],
                        in1=xpbt_sb[:, it, :],
                        op0=mybir.AluOpType.mult,
                        op1=mybir.AluOpType.add,
                    )
                else:
                    # scalar (PSUM-capable) normalizes; pool adds the residual
                    at = apool.tile([128, 512], F32, tag="at")
                    nc.scalar.activation(
                        at[:],
                        ps[:],
                        mybir.ActivationFunctionType.Identity,
                        scale=rcol_sb[:, it : it + 1],
                    )
                    nc.gpsimd.tensor_add(yt[:], at[:], xpbt_sb[:, it, :])
                nc.sync.dma_start(y_d[:, it, :], yt[:])

            for jt in range(JT):
                s_tile(jt, 0)
                u_tile(jt)
            for k in range(4):
                s_tile(2 * k, 1)
                s_tile(2 * k + 1, 1)
                den_col(k)
                out_tile(k)
            for it in (4, 5, 6, 7):
                den_col(it)
                out_tile(it)

    nc.compile()
    return nc


def kernel(x, gn_weight, gn_bias, qkv_w, qkv_b, proj_w, proj_b):
    global LAST_RESULTS
    b, c, h, w = x.shape
    assert (b, c, h * w) == (8, C, NPIX)

    qkv_b = np.asarray(qkv_b, np.float64)
    qkv_w = np.asarray(qkv_w, np.float64)
    proj_w = np.asarray(proj_w, np.float64)
    proj_b = np.asarray(proj_b, np.float64)

    if "nc" not in _cache:
        _cache["nc"] = _build()
    nc = _cache["nc"]

    def col(v):  # [512] vector -> [128, CT] per-partition columns
        return np.asarray(v, np.float32).reshape(CT, 128).T

    def wtile(wT):  # [c_in, cols] fp8 -> [128, CT, cols]
        return np.ascontiguousarray(
            wT.reshape(CT, 128, wT.shape[1]).transpose(1, 0, 2)
        )

    Wq, Wk, Wv = qkv_w[0:C], qkv_w[C : 2 * C], qkv_w[2 * C :]
    A = Wq.T @ Wk                     # q/k fold: logits = xn^T A xn (+ w0^T xn_j)
    W_pv = proj_w @ Wv                # proj fused into v
    # per-key logit shift from the q bias; x WSCALE matches the t PSUM scale
    w0 = WSCALE * (Wk.T @ qkv_b[0:C])
    pb = proj_b + proj_w @ qkv_b[2 * C :]  # v bias folds through attention

    qw_host = np.concatenate(
        [(WSCALE * A).astype(NPF8), (WSCALE * W_pv.T).astype(NPF8)], axis=1
    )  # [c_in, 2C] fp8

    cols_host = np.stack(
        [col(gn_weight), col(gn_bias), col(w0)], axis=2
    )  # [128, CT, 3]

    shared = {
        "qw": wtile(qw_host),
        "cols": np.ascontiguousarray(cols_host),
    }
    xf = np.asarray(x, np.float32).reshape(b, C, NPIX)
    xs = xf.reshape(b, CT, 128, NPIX)
    pb32 = pb.astype(np.float32)
    in_maps = [
        {
            "x": np.ascontiguousarray(
                xs[i].transpose(1, 0, 2).astype(ml_dtypes.bfloat16)
            ),
            # residual in the output (transposed) layout, pb pre-added
            "xpbt": np.ascontiguousarray(
                (xf[i].reshape(C, JT, 128).transpose(2, 1, 0)
                 + pb32[None, None, :]).astype(ml_dtypes.bfloat16)
            ),
            **shared,
        }
        for i in range(b)
    ]

    res = run_bass_kernel_spmd(
        nc, in_maps, core_ids=list(range(8)), trace=TRACE, **TRACE_KW
    )
    LAST_RESULTS = res
    out = np.stack(
        [r["y"].transpose(2, 1, 0).reshape(c, h, w) for r in res.results]
    )
    return out.astype(np.float32)


# revision 24
# speedup vs baseline: 1.1752x; 1.1752x over previous
"""AttentionBlock (GroupNorm + single-head spatial attention + proj + residual)
on 8 trn2 NeuronCores, data-parallel over the batch (1 image per core).

Full inputs in, full outputs out. v2: all GEMMs run in fp8e4m3 with DoubleRow
perf mode (2 contraction rows/cycle), and the proj matmul is fused into the
value projection host-side:

    y = x + proj_b' + (W_pv xn) E D^-1,   W_pv = proj_w @ W_v,
    E = exp(scale * xn^T A xn),           A = W_q^T W_k  (q/k fold),
    D = colsum(E).

Bias handling is exact for arbitrary qkv_b: the v-bias folds through the
attention (rows sum to 1) into proj_b'; the q-bias contributes a per-key
logit term w0^T xn_j with w0 = W_k^T q_b, added as the per-partition bias of
the t-evacuation; the k-bias only produces per-query logit shifts, which
softmax cancels.

fp8 precision (measured host-side vs the fp32 reference): rel err ~4.5e-3,
dominated by the E/value quantization; the residual branch is only ~7% of
the output norm. A and W_pv are pre-scaled by 16 to dodge the fp8 subnormal
cutoff; the 1/16 rides the PSUM-evacuation activation scale.

Layouts (prepared host-side so every DMA is contiguous, no device transposes):
  - activations [128 part, ct, pix] (channel tiles of 128)
  - qw = [A*16 | W_pv^T*16] as [c_in part, ct_in, 2C] fp8
  - u is produced transposed (u^T = xn^T W_pv^T) so the attention contraction
    over pixels has pixels on partitions everywhere.
"""

import sys

sys.path.insert(0, "/opt/trn_rl_repo")

import numpy as np
import ml_dtypes

import concourse.bass as bass
import concourse.tile as tile
from concourse import bacc, mybir
from concourse.bass_utils import run_bass_kernel_spmd
from concourse.tile_rust import add_dep_helper

F32 = mybir.dt.float32
FP8 = mybir.dt.float8e4
NPF8 = ml_dtypes.float8_e4m3
DR = mybir.MatmulPerfMode.DoubleRow

C = 512          # channels
NPIX = 1024      # pixels per image (32*32)
CT = 4           # channel tiles of 128
JT = 8           # pixel tiles of 128
NH = 2           # halves of NPIX for the 512-wide moving dim
G = 32           # groups
GS = 16          # channels per group
EPS = 1e-5
SCALE = C ** -0.5
WSCALE = 16.0    # host pre-scale on A / W_pv to stay clear of fp8 subnormals

WARM_MMS = 27    # PE warm-up matmuls covering the input-DMA + GN-stats window
FILL1 = 5        # keep-warm matmuls between the GN group-sum and broadcast
FILL2 = 13       # keep-warm matmuls covering the xn-quantize window

TRACE = False          # set True (from test.py) to capture an NTFF profile
TRACE_KW = {}          # extra kwargs for run_bass_kernel_spmd
LAST_RESULTS = None    # BassKernelResults of the most recent run

_cache = {}


def _build():
    nc = bacc.Bacc("TRN2")

    # x ships as bf16: it only feeds the GN stats and the fp8 quantize;
    # the residual rides xpbt. Halves the critical input-DMA window.
    x_d = nc.dram_tensor("x", [128, CT, NPIX], mybir.dt.bfloat16, kind="ExternalInput")
    qw_d = nc.dram_tensor("qw", [128, CT, 2 * C], FP8, kind="ExternalInput")
    # packed per-channel columns: gnw, gnb, w0
    cols_d = nc.dram_tensor("cols", [128, CT, 3], F32, kind="ExternalInput")
    # x^T + pb, [pix part, jt, c] — the residual in the output layout (bf16)
    BF16 = mybir.dt.bfloat16
    xpbt_d = nc.dram_tensor("xpbt", [128, JT, C], BF16, kind="ExternalInput")
    # y^T [pix part, jt, c]; host transposes back
    y_d = nc.dram_tensor("y", [128, JT, C], F32, kind="ExternalOutput")

    # Indicator constants for the cross-partition group reductions.
    # ind1[p, ct*G + g] = 1 iff channel (ct*128+p) belongs to group g.
    ind1 = np.zeros((128, CT * G), np.float32)
    for ct in range(CT):
        for p in range(128):
            ind1[p, ct * G + ct * 8 + p // GS] = 1.0
    # ind2[g, c] = 1 iff channel c belongs to group g.
    ind2 = np.zeros((G, C), np.float32)
    for c in range(C):
        ind2[c // GS, c] = 1.0
    ind1_d = nc.inline_tensor(ind1, name="ind1")
    ind2_d = nc.inline_tensor(ind2, name="ind2")

    with tile.TileContext(nc) as tc:
        with (
            nc.allow_low_precision(reason="fp8 matmuls, fp8 activation stores"),
            tc.tile_pool(name="persist", bufs=1) as pers,
            tc.tile_pool(name="small", bufs=4) as spool,
            tc.tile_pool(name="att", bufs=4) as apool,
            tc.tile_pool(name="ps", bufs=8, space="PSUM") as psp,
        ):
            # ---- x, one DMA per (ct, half); issues split across the two
            # HWDGE-capable queues (sync + scalar) so more DMA rings engage.
            x_sb = pers.tile([128, CT, NPIX], mybir.dt.bfloat16)
            x_dmas = []
            for ct in range(CT):
                for nh in range(NH):
                    eng = nc.sync if (ct * NH + nh) % 2 == 0 else nc.scalar
                    x_dmas.append(
                        eng.dma_start(
                            x_sb[:, ct, nh * 512 : (nh + 1) * 512],
                            x_d[:, ct, nh * 512 : (nh + 1) * 512],
                        )
                    )

            # ---- small loads (after x in the issue queue) ------------------
            cols_sb = pers.tile([128, CT, 3], F32)
            nc.sync.dma_start(cols_sb[:], cols_d[:])
            ind1_sb = pers.tile([128, CT * G], F32)
            nc.sync.dma_start(ind1_sb[:], ind1_d[:])
            ind2_sb = pers.tile([G, C], F32)
            nc.sync.dma_start(ind2_sb[:], ind2_d[:])

            # ---- weights + residual, serialized behind x -------------------
            qw_sb = pers.tile([128, CT, 2 * C], FP8)
            d = nc.sync.dma_start(qw_sb[:], qw_d[:])
            add_dep_helper(d.ins, x_dmas[-1].ins, sync=True,
                           reason="let x DMA finish first")
            xpbt_sb = pers.tile([128, JT, C], BF16)
            d = nc.sync.dma_start(xpbt_sb[:], xpbt_d[:])
            add_dep_helper(d.ins, x_dmas[-1].ins, sync=True,
                           reason="let x DMA finish first")

            gnw_sb = cols_sb[:, :, 0]
            gnb_sb = cols_sb[:, :, 1]
            w0_sb = cols_sb[:, :, 2]

            # ---- constants from memsets (no DMA dependency) ----------------
            warm8 = pers.tile([128, 2, 512], FP8)
            nc.gpsimd.memset(warm8[:], 1.0)
            eps_sb = pers.tile([G, 1], F32)
            nc.vector.memset(eps_sb[:], EPS)
            # pre-load the Exp activation table during the DMA window so the
            # scalar engine doesn't stall the S phase with a mid-stream load
            dume = pers.tile([1, 1], FP8)
            duml = pers.tile([1, 1], F32)
            nc.scalar.activation(
                duml[:], eps_sb[0:1, 0:1], mybir.ActivationFunctionType.Ln
            )
            nc.scalar.activation(
                dume[:], eps_sb[0:1, 0:1], mybir.ActivationFunctionType.Exp,
                scale=SCALE / WSCALE,
            )

            def warm(n):
                for _ in range(n):
                    wps = psp.tile([128, 512], F32, tag="ps")
                    nc.tensor.matmul(
                        wps[:], warm8[:, :, 0:128], warm8[:], start=True,
                        stop=True, perf_mode=DR,
                    )

            # ---- PE warm-up: ramp the clock while the input DMAs stream ----
            warm(WARM_MMS)

            # ---- group norm ------------------------------------------------
            # per-channel mean / E[x^2] along pixels, then group-combine via
            # indicator matmuls (contraction over the partition dim).
            statcols = pers.tile([128, CT, 2], F32)
            for ct in range(CT):
                st6 = spool.tile([128, 2, 6], F32, tag="st6")
                nc.vector.bn_stats(st6[:, 0, :], x_sb[:, ct, 0:512])
                nc.vector.bn_stats(st6[:, 1, :], x_sb[:, ct, 512:1024])
                mv = spool.tile([128, 2], F32, tag="mv")
                nc.vector.bn_aggr(mv[:], st6[:])
                nc.vector.tensor_copy(statcols[:, ct, 0:1], mv[:, 0:1])
                # E[x^2] = var + mean^2
                nc.vector.tensor_mul(statcols[:, ct, 1:2], mv[:, 0:1], mv[:, 0:1])
                nc.vector.tensor_add(
                    statcols[:, ct, 1:2], statcols[:, ct, 1:2], mv[:, 1:2]
                )

            gsum_ps = psp.tile([G, 2], F32, tag="ps")
            for ct in range(CT):
                nc.tensor.matmul(
                    gsum_ps[:],
                    ind1_sb[:, ct * G : (ct + 1) * G],
                    statcols[:, ct, :],
                    start=(ct == 0),
                    stop=(ct == CT - 1),
                )
            warm(FILL1)
            gs_sb = spool.tile([G, 2], F32, tag="gs")
            nc.vector.tensor_scalar_mul(gs_sb[:], gsum_ps[:], 1.0 / GS)
            var32 = spool.tile([G, 1], F32, tag="var32")
            nc.vector.tensor_mul(var32[:], gs_sb[:, 0:1], gs_sb[:, 0:1])
            nc.vector.tensor_sub(var32[:], gs_sb[:, 1:2], var32[:])
            # grow = [rstd, mean * rstd] per group;
            # rstd = exp(-0.5*ln(var+eps)) keeps scalar on one act table.
            grow = pers.tile([G, 2], F32)
            lnv = spool.tile([G, 1], F32, tag="lnv")
            nc.scalar.activation(
                lnv[:],
                var32[:],
                mybir.ActivationFunctionType.Ln,
                bias=eps_sb[:],
            )
            nc.scalar.activation(
                grow[:, 0:1],
                lnv[:],
                mybir.ActivationFunctionType.Exp,
                scale=-0.5,
            )
            nc.vector.tensor_mul(grow[:, 1:2], gs_sb[:, 0:1], grow[:, 0:1])

            # broadcast group stats back to channels; fold gn weight/bias into
            # per-channel scale A and bias B:  xn = x*A + B
            chsb = pers.tile([128, CT, 2], F32)
            for ct in range(CT):
                bc_ps = psp.tile([128, 2], F32, tag="ps")
                nc.tensor.matmul(
                    bc_ps[:],
                    ind2_sb[:, ct * 128 : (ct + 1) * 128],
                    grow[:],
                    start=True,
                    stop=True,
                )
                nc.vector.tensor_mul(
                    chsb[:, ct, 0:1], gnw_sb[:, ct : ct + 1], bc_ps[:, 0:1]
                )
                nc.vector.tensor_mul(
                    chsb[:, ct, 1:2], gnw_sb[:, ct : ct + 1], bc_ps[:, 1:2]
                )
                nc.vector.tensor_sub(
                    chsb[:, ct, 1:2], gnb_sb[:, ct : ct + 1], chsb[:, ct, 1:2]
                )
            warm(FILL2)

            # ---- xn in fp8, nh-major so the first GEMMs start early.
            # Split vector/pool: both halves in flight at once.
            xn8 = pers.tile([128, CT, NPIX], FP8)
            for nh in range(NH):
                for ct in range(CT):
                    sl = (slice(None), ct, slice(nh * 512, (nh + 1) * 512))
                    eng = nc.vector if (ct % 2 == 0) else nc.gpsimd
                    eng.tensor_scalar(
                        out=xn8[sl],
                        in0=x_sb[sl],
                        scalar1=chsb[:, ct, 0:1],
                        scalar2=chsb[:, ct, 1:2],
                        op0=mybir.AluOpType.mult,
                        op1=mybir.AluOpType.add,
                    )

            # ---- t = A^T xn (+ w0), fp8; evac on scalar --------------------
            t8 = pers.tile([128, CT, NPIX], FP8)
            for nh in range(NH):
                for co in range(CT):
                    ps = psp.tile([128, 512], F32, tag="ps")
                    for cp in range(2):
                        nc.tensor.matmul(
                            ps[:],
                            qw_sb[:, 2 * cp : 2 * cp + 2, co * 128 : (co + 1) * 128],
                            xn8[:, 2 * cp : 2 * cp + 2, nh * 512 : (nh + 1) * 512],
                            start=(cp == 0),
                            stop=(cp == 1),
                            perf_mode=DR,
                        )
                    nc.vector.tensor_scalar(
                        out=t8[:, co, nh * 512 : (nh + 1) * 512],
                        in0=ps[:],
                        scalar1=w0_sb[:, co : co + 1],
                        scalar2=None,
                        op0=mybir.AluOpType.add,
                    )

            e8 = pers.tile([128, JT, NPIX], FP8)
            rcol_sb = pers.tile([128, JT], F32)
            u8 = pers.tile([128, JT, C], FP8)

            # The scalar engine (24 exp/copy evacuations) paces the t/S
            # phases, so u, den and out matmuls are interleaved into the S
            # stream to keep the PE busy during scalar waits.

            def s_tile(jt, nh):
                # S^T = xn^T t (pix_j on partitions), E = exp(scale * S^T)
                ps = psp.tile([128, 512], F32, tag="ps")
                for cp in range(2):
                    nc.tensor.matmul(
                        ps[:],
                        xn8[:, 2 * cp : 2 * cp + 2, jt * 128 : (jt + 1) * 128],
                        t8[:, 2 * cp : 2 * cp + 2, nh * 512 : (nh + 1) * 512],
                        start=(cp == 0),
                        stop=(cp == 1),
                        perf_mode=DR,
                    )
                nc.scalar.activation(
                    e8[:, jt, nh * 512 : (nh + 1) * 512],
                    ps[:],
                    mybir.ActivationFunctionType.Exp,
                    scale=SCALE / WSCALE,
                )

            def u_tile(jt):
                # u^T = xn^T W_pv^T, fp8; evac on vector
                ps = psp.tile([128, 512], F32, tag="ps")
                for cp in range(2):
                    nc.tensor.matmul(
                        ps[:],
                        xn8[:, 2 * cp : 2 * cp + 2, jt * 128 : (jt + 1) * 128],
                        qw_sb[:, 2 * cp : 2 * cp + 2, C : 2 * C],
                        start=(cp == 0),
                        stop=(cp == 1),
                        perf_mode=DR,
                    )
                nc.vector.tensor_scalar_mul(u8[:, jt, :], ps[:], 1.0 / WSCALE)

            def den_col(it):
                # den[i] = sum_j E[j, i] as a per-partition column: E-tile as
                # stationary, fp8 ones as the (2-row) moving operand.
                dps = psp.tile([128, 1], F32, name=f"den{it}", tag="ps")
                for p in range(4):
                    nc.tensor.matmul(
                        dps[:],
                        e8[:, 2 * p : 2 * p + 2, it * 128 : (it + 1) * 128],
                        warm8[:, 0:2, 0:1],
                        start=(p == 0),
                        stop=(p == 3),
                        perf_mode=DR,
                    )
                rscr = spool.tile([128, 1], F32, tag="rscr")
                nc.vector.reciprocal_approx_accurate(
                    rcol_sb[:, it : it + 1], dps[:], rscr[:]
                )

            def out_tile(it):
                # out^T = (u E)^T D^-1 + (x^T + pb): E-tile stationary puts
                # queries on partitions, so 1/den is a per-partition scalar
                # on the (single, fused) vector evacuation.
                ps = psp.tile([128, 512], F32, tag="ps")
                for p in range(4):
                    nc.tensor.matmul(
                        ps[:],
                        e8[:, 2 * p : 2 * p + 2, it * 128 : (it + 1) * 128],
                        u8[:, 2 * p : 2 * p + 2, :],
                        start=(p == 0),
                        stop=(p == 3),
                        perf_mode=DR,
                    )
                yt = apool.tile([128, 512], F32, tag="yt")
                if it % 2 == 1:
                    nc.vector.scalar_tensor_tensor(
                        out=yt[:],
                        in0=ps[:],
                        scalar=rcol_sb[:, it : it + 1],
                        in1=xpbt_sb[:, it, :],
                        op0=mybir.AluOpType.mult,
                        op1=mybir.AluOpType.add,
                    )
                else:
                    # scalar (PSUM-capable) normalizes; pool adds the residual
                    at = apool.tile([128, 512], F32, tag="at")
                    nc.scalar.activation(
                        at[:],
                        ps[:],
                        mybir.ActivationFunctionType.Identity,
                        scale=rcol_sb[:, it : it + 1],
                    )
                    nc.gpsimd.tensor_add(yt[:], at[:], xpbt_sb[:, it, :])
                nc.sync.dma_start(y_d[:, it, :], yt[:])

            for jt in range(JT):
                s_tile(jt, 0)
                u_tile(jt)
            for k in range(4):
                s_tile(2 * k, 1)
                s_tile(2 * k + 1, 1)
                den_col(k)
                out_tile(k)
            for it in (4, 5, 6, 7):
                den_col(it)
                out_tile(it)

    nc.compile()
    return nc


def kernel(x, gn_weight, gn_bias, qkv_w, qkv_b, proj_w, proj_b):
    global LAST_RESULTS
    b, c, h, w = x.shape
    assert (b, c, h * w) == (8, C, NPIX)

    qkv_b = np.asarray(qkv_b, np.float64)
    qkv_w = np.asarray(qkv_w, np.float64)
    proj_w = np.asarray(proj_w, np.float64)
    proj_b = np.asarray(proj_b, np.float64)

    if "nc" not in _cache:
        _cache["nc"] = _build()
    nc = _cache["nc"]

    def col(v):  # [512] vector -> [128, CT] per-partition columns
        return np.asarray(v, np.float32).reshape(CT, 128).T

    def wtile(wT):  # [c_in, cols] fp8 -> [128, CT, cols]
        return np.ascontiguousarray(
            wT.reshape(CT, 128, wT.shape[1]).transpose(1, 0, 2)
        )

    Wq, Wk, Wv = qkv_w[0:C], qkv_w[C : 2 * C], qkv_w[2 * C :]
    A = Wq.T @ Wk                     # q/k fold: logits = xn^T A xn (+ w0^T xn_j)
    W_pv = proj_w @ Wv                # proj fused into v
    # per-key logit shift from the q bias; x WSCALE matches the t PSUM scale
    w0 = WSCALE * (Wk.T @ qkv_b[0:C])
    pb = proj_b + proj_w @ qkv_b[2 * C :]  # v bias folds through attention

    qw_host = np.concatenate(
        [(WSCALE * A).astype(NPF8), (WSCALE * W_pv.T).astype(NPF8)], axis=1
    )  # [c_in, 2C] fp8

    cols_host = np.stack(
        [col(gn_weight), col(gn_bias), col(w0)], axis=2
    )  # [128, CT, 3]

    shared = {
        "qw": wtile(qw_host),
        "cols": np.ascontiguousarray(cols_host),
    }
    xf = np.asarray(x, np.float32).reshape(b, C, NPIX)
    xs = xf.reshape(b, CT, 128, NPIX)
    pb32 = pb.astype(np.float32)
    in_maps = [
        {
            "x": np.ascontiguousarray(
                xs[i].transpose(1, 0, 2).astype(ml_dtypes.bfloat16)
            ),
            # residual in the output (transposed) layout, pb pre-added
            "xpbt": np.ascontiguousarray(
                (xf[i].reshape(C, JT, 128).transpose(2, 1, 0)
                 + pb32[None, None, :]).astype(ml_dtypes.bfloat16)
            ),
            **shared,
        }
        for i in range(b)
    ]

    res = run_bass_kernel_spmd(
        nc, in_maps, core_ids=list(range(8)), trace=TRACE, **TRACE_KW
    )
    LAST_RESULTS = res
    out = np.stack(
        [r["y"].transpose(2, 1, 0).reshape(c, h, w) for r in res.results]
    )
    return out.astype(np.float32)
